# revision 40
# baseline (speedup 1.0000x reference)
"""Causal multi-head attention (B=2, H=16, S=2048, D=128, fp32) on 8 NeuronCores.

Sharding: the 32 (batch, head) pairs are split 4-per-core (tensor parallel over
heads, data parallel over batch — both collapse to the fused pair axis).

Per-core kernel, flash-attention style without max-subtraction (scores have
unit variance after the 1/sqrt(D) scale).  All exponentials carry a uniform
shift exp(s - CSHIFT), which softmax normalization cancels.

One flat pipeline over 96 score strips (pair, chunk, k-block).  The exp of
each strip is column-split across BOTH elementwise engines, and each engine
has its OWN score PSUM rotation so neither can stall the other:

  scores_T[k, 0:512]    -> scA (1-bank PSUM, pool A)   exp'd by ScalarE (Exp)
  scores_T[k, 512:1024] -> scB (1-bank PSUM, pool B)   exp'd by DVE via a
      Schraudolph bit-trick: t = rne_int16(s*A + B); bitcast(t) is bf16
      ~ exp(s*SCALE - C) with ~3.3% max rel err (mean-free after softmax)

  pool A's buffer rotation depends only on ScalarE's exp, pool B's only on
  DVE's, and PE's in-order queue is arranged as
      [scores_A(i+1)] [PV batch(i-1)] [scores_B(i+1)]
  so the PV matmuls (which by then have no unresolved deps) stream at full
  rate and a lag in either exp engine delays only its own half.

  causal mask on diagonal 128x128 blocks     (DVE multiply by a const mask)
  ctx[q, 0:128], l[q] = P_T_blk.T @ [V | 1]  (bf16 matmuls, PSUM-accumulated;
                                              the ones column gives the
                                              softmax denominator for free;
                                              emitted PV_DELAY strips late so
                                              they never carry unresolved
                                              deps into PE's in-order queue)
  out[q, :] = ctx[q, :] / l[q]               (DVE batched reciprocal per PSUM
                                              bank + ScalarE Copy-with-scale)

All input DMAs are issued up-front on the (otherwise idle) GPSIMD trigger
queue in consumption order; output DMAs go on the Sync queue.  PSUM layout is
exactly 8 banks: scA x2, scB x2, ctx0, ctx1, ctx2 x2 (double-buffered).
PSUM start=True clears has_written for a whole bank, so of the 8 packed ctx
accumulation groups only the first per bank (s = 0/3/6 at kb==0) uses it.

Q^T / K^T (bf16) and the bf16 [V | 1] augmentation are prepared host-side in
kernel() — host preprocessing is part of the sharding step.
"""

import math

import ml_dtypes
import numpy as np

import concourse.bass as bass
import concourse.mybir as mybir
from concourse import bacc, tile
from concourse.bass_utils import run_bass_kernel_spmd

B, H, S, D = 2, 16, 2048, 128
NCORES = 8
NPAIRS = B * H              # 32 fused (batch, head) pairs
PPC = NPAIRS // NCORES      # 4 pairs per core
KB = 128                    # k block (PE contraction / partition dim)
QC = 1024                   # q chunk (2 half-strips)
HC = 512                    # half-strip columns (one PSUM bank)
NSUB = QC // 128            # sub-q blocks (PV stationary width) per chunk
NKT = S // KB               # 16 k blocks per sequence
SCALE = 1.0 / math.sqrt(D)  # net score scale: /(sqrt(d)*coeff) then *coeff
CSHIFT = 1.25               # uniform exponent shift (cancels in softmax)

# Schraudolph constants: bf16(int16_rne(s_raw*A + B)) ~ exp(s_raw*SCALE - C)
_DELTA = math.log2((1 + (1 / math.log(2) - 1)) / 2 ** (1 / math.log(2) - 1)) / 2
A_SCH = 128 * math.log2(math.e) * SCALE
B_SCH = 128 * 127 - 128 * _DELTA - CSHIFT * 128 * math.log2(math.e)

# ctx bank groups normalized on ScalarE instead of DVE (bank index 0/1/2)
NORM_ACT_BANKS = {0, 1}

F32 = mybir.dt.float32
BF16 = mybir.dt.bfloat16
I16 = mybir.dt.int16


def _build_nc():
    nc = bacc.Bacc("TRN2", target_bir_lowering=False, debug=False)
    qt_d = nc.dram_tensor("qt", [PPC, D, S], BF16, kind="ExternalInput")
    kt_d = nc.dram_tensor("kt", [PPC, D, S], BF16, kind="ExternalInput")
    va_d = nc.dram_tensor("va", [PPC, KB, NKT, KB + 1], BF16, kind="ExternalInput")
    out_d = nc.dram_tensor("out", [PPC, S, D], F32, kind="ExternalOutput")

    # Raw-bass warmup activation before the Tile body: bacc's table-load
    # placement then puts the ~1.3us ACT table load in the preamble, off the
    # first strip's critical path. Persistent scratch; address never reused.
    warm_sb = nc.alloc_sbuf_tensor("warm_sb", [128, 1], F32)
    nc.scalar.activation(
        warm_sb.ap(), warm_sb.ap(), mybir.ActivationFunctionType.Exp, scale=0.0
    )

    # chunk order per pair: last pair does its big chunk first so the kernel
    # tail is the small chunk's short PV backlog
    qcs_of = [[0, 1] if p < PPC - 1 else [1, 0] for p in range(PPC)]

    def nkb_of(qc):
        return (qc * QC + QC) // KB

    with tile.TileContext(nc) as tc:
        with (
            tc.tile_pool(name="cm", bufs=1) as c_pool,
            tc.tile_pool(name="qk", bufs=3) as qk_pool,
            tc.tile_pool(name="vp", bufs=3) as v_pool,
            tc.tile_pool(name="pp", bufs=6) as p_pool,
            tc.tile_pool(name="oo", bufs=8) as o_pool,
            tc.tile_pool(name="rr", bufs=8) as r_pool,
            tc.tile_pool(name="ps_a", bufs=2, space="PSUM") as ps_a,
            tc.tile_pool(name="ps_b", bufs=2, space="PSUM") as ps_b,
            tc.tile_pool(name="ps_c", bufs=1, space="PSUM") as ps_c,
            tc.tile_pool(name="ps_c2", bufs=2, space="PSUM") as ps_c2,
        ):
            # exp bias lives on the DVE preamble so the gpsimd queue can
            # start input DMAs immediately; the causal mask tile is built
            # after pair 0's first DMA pieces (it is first read much later)
            mask_t = c_pool.tile([KB, KB], BF16, name="mask_t")
            bias_t = c_pool.tile([KB, 1], F32, name="bias_t")
            nc.vector.memset(bias_t[:], -CSHIFT)

            def build_mask():
                nc.gpsimd.memset(mask_t[:], 1.0)
                nc.gpsimd.affine_select(
                    out=mask_t[:],
                    in_=mask_t[:],
                    compare_op=mybir.AluOpType.is_ge,
                    fill=0.0,
                    base=0,
                    pattern=[[1, KB]],
                    channel_multiplier=-1,
                )

            # all input DMAs up-front on the gpsimd trigger queue, in
            # consumption order (the queue blocks on pool-buffer reuse, which
            # is fine — nothing else runs on gpsimd)
            pair_tiles = []
            for p in range(PPC):
                qt_t = qk_pool.tile([D, S], BF16, tag="qt", name="qt_t")
                kt_t = qk_pool.tile([D, S], BF16, tag="kt", name="kt_t")
                va_t = v_pool.tile([KB, NKT, KB + 1], BF16, tag="va", name="va_t")
                if p == 0:
                    # fine-grained staging so the first scores matmuls (which
                    # need only kt[:, 0:256] / qt[:, 0:1024]) start ASAP; the
                    # first pieces ride the Sync queue, which finishes its
                    # preamble while gpsimd is still building masks
                    qcp = qcs_of[0][0]
                    q0 = qcp * QC
                    nc.sync.dma_start(
                        out=kt_t[:, 0:2 * KB], in_=kt_d[p][:, 0:2 * KB]
                    )
                    nc.sync.dma_start(
                        out=qt_t[:, q0:q0 + QC], in_=qt_d[p][:, q0:q0 + QC]
                    )
                    nc.sync.dma_start(
                        out=kt_t[:, 2 * KB:QC], in_=kt_d[p][:, 2 * KB:QC]
                    )
                    nc.sync.dma_start(
                        out=va_t[:, 0:4], in_=va_d[p][:, 0:4]
                    )
                    if qcp == 0:
                        nc.gpsimd.dma_start(out=qt_t[:, QC:], in_=qt_d[p][:, QC:])
                    nc.gpsimd.dma_start(out=kt_t[:, QC:], in_=kt_d[p][:, QC:])
                    nc.gpsimd.dma_start(out=va_t[:, 4:], in_=va_d[p][:, 4:])
                    build_mask()
                    pair_tiles.append((qt_t, kt_t, va_t))
                    continue
                for qcp in qcs_of[p]:
                    c0, c1 = qcp * QC, (qcp + 1) * QC
                    nc.gpsimd.dma_start(out=kt_t[:, c0:c1], in_=kt_d[p][:, c0:c1])
                    nc.gpsimd.dma_start(out=qt_t[:, c0:c1], in_=qt_d[p][:, c0:c1])
                if qcs_of[p][0] == 0:
                    kbm = QC // KB
                    nc.gpsimd.dma_start(out=va_t[:, 0:kbm], in_=va_d[p][:, 0:kbm])
                    nc.gpsimd.dma_start(out=va_t[:, kbm:], in_=va_d[p][:, kbm:])
                else:
                    nc.gpsimd.dma_start(out=va_t[:], in_=va_d[p])
                pair_tiles.append((qt_t, kt_t, va_t))

            plan = [
                (p, qc, kb)
                for p in range(PPC)
                for qc in qcs_of[p]
                for kb in range(nkb_of(qc))
            ]

            def lo_of(qc, kb):
                return max(kb * KB - qc * QC, 0)

            def emit_scores_half(p, qc, kb, half):
                """Scores MM for one 512-col half; returns its PSUM tile
                (None if the half is fully causally dead)."""
                lo = lo_of(qc, kb)
                c0, c1 = half * HC, half * HC + HC
                if lo >= c1:
                    return None
                qt_t, kt_t, _ = pair_tiles[p]
                q0 = qc * QC
                k0 = kb * KB
                pool = ps_a if half == 0 else ps_b
                sc = pool.tile([KB, HC], F32, tag="sc", name="sc")
                cc = max(c0, lo)
                nc.tensor.matmul(
                    sc[:, cc - c0:],
                    kt_t[:, k0:k0 + KB],
                    qt_t[:, q0 + cc:q0 + c1],
                    start=True,
                    stop=True,
                )
                return sc

            def emit_pv_and_norm(rec):
                # PV matmuls + normalize for a strip, emitted one strip LATE:
                # by now its exp/mask are long finished, so these matmuls
                # never stall PE's in-order queue ahead of the next scores.
                p, qc, kb, pt, ctx_tiles, va_t = rec
                q0 = qc * QC
                off = kb * KB - q0

                def ctx_ap(s):
                    t, ii = divmod(s, 3)
                    return ctx_tiles[t][:, ii, :]

                s_order = [s for s in range(NSUB) if off <= s * 128]
                if off >= 0 and kb > 0 and s_order[0] * 128 == off:
                    s_order = s_order[1:] + s_order[:1]
                for s in s_order:
                    qs0 = s * 128
                    nc.tensor.matmul(
                        ctx_ap(s),
                        pt[:, qs0:qs0 + 128],
                        va_t[:, kb, :],
                        start=(kb == 0 and s % 3 == 0),
                        stop=(kb == q0 // KB + s),
                        skip_group_check=True,
                    )
                for bank, s_hi in ((0, 2), (1, 5), (2, 7)):
                    if kb != q0 // KB + s_hi:
                        continue
                    s_lo = 3 * bank
                    nsb = s_hi - s_lo + 1
                    ob = o_pool.tile([128, 3, D], F32, tag="ob", name="ob")
                    rec_t = r_pool.tile([128, 3], F32, tag="rec", name="rec_t")
                    nc.vector.reciprocal(
                        rec_t[:, 0:nsb], ctx_tiles[bank][:, 0:nsb, D]
                    )
                    if bank in NORM_ACT_BANKS:
                        for s in range(s_lo, s_hi + 1):
                            j = s - s_lo
                            nc.scalar.activation(
                                ob[:, j, :],
                                ctx_ap(s)[:, 0:D],
                                mybir.ActivationFunctionType.Copy,
                                scale=rec_t[:, j:j + 1],
                            )
                    else:
                        nc.vector.scalar_tensor_tensor(
                            out=ob[:, 0:nsb, :],
                            in0=ctx_tiles[bank][:, 0:nsb, 0:D],
                            scalar=1.0,
                            in1=rec_t[:, 0:nsb].to_broadcast((128, nsb, D)),
                            op0=mybir.AluOpType.mult,
                            op1=mybir.AluOpType.mult,
                        )
                    nc.sync.dma_start(
                        out=out_d[
                            p, q0 + s_lo * 128:q0 + (s_hi + 1) * 128, :
                        ].rearrange("(s q) d -> q s d", s=nsb),
                        in_=ob[:, 0:nsb, :],
                    )

            scA = emit_scores_half(*plan[0], 0)
            scB = emit_scores_half(*plan[0], 1)
            ctx_tiles = None
            pending = []  # PV batches run PV_DELAY strips late
            PV_DELAY = 5
            small_flip = 0
            for i, (p, qc, kb) in enumerate(plan):
                q0 = qc * QC
                off = kb * KB - q0  # >= 0 on diagonal strips
                lo = max(off, 0)
                if kb == 0:
                    # 8 ctx accumulators [128q, 129] for this chunk, packed
                    # 3/3/2 into PSUM banks
                    ctx_tiles = [
                        ps_c.tile([128, 3, KB + 1], F32, tag="ctx0", name="ctx0"),
                        ps_c.tile([128, 3, KB + 1], F32, tag="ctx1", name="ctx1"),
                        ps_c2.tile([128, 2, KB + 1], F32, tag="ctx2", name="ctx2"),
                    ]

                pt = p_pool.tile([KB, QC], BF16, tag="pt", bufs=10, name="pt")

                def exp_act(sc, g0, g1, pt=pt):
                    # ScalarE exp of global cols [g0:g1) from half tile sc
                    h0 = (g0 // HC) * HC
                    nc.scalar.activation(
                        pt[:, g0:g1],
                        sc[:, g0 - h0:g1 - h0],
                        mybir.ActivationFunctionType.Exp,
                        scale=SCALE,
                        bias=bias_t[:],
                    )

                def exp_dve(sc, g0, g1, pt=pt):
                    h0 = (g0 // HC) * HC
                    nc.vector.tensor_scalar(
                        out=pt[:, g0:g1].bitcast(I16),
                        in0=sc[:, g0 - h0:g1 - h0],
                        scalar1=A_SCH,
                        scalar2=B_SCH,
                        op0=mybir.AluOpType.mult,
                        op1=mybir.AluOpType.add,
                    )

                if lo < HC:
                    exp_act(scA, lo, HC)
                    exp_dve(scB, HC, QC)
                elif small_flip == 0:
                    exp_act(scB, lo, QC)
                    small_flip = 1
                else:
                    exp_dve(scB, lo, QC)
                    small_flip = 0

                # next strip's half-A scores feed ScalarE as soon as its pool
                # slot frees; half-B goes after the PV batch so a DVE lag
                # never blocks the PV stream in PE's in-order queue
                if i + 1 < len(plan):
                    scA = emit_scores_half(*plan[i + 1], 0)

                if off >= 0:
                    # diagonal 128x128 block: keep j >= i, zero rest
                    nc.vector.tensor_mul(
                        pt[:, off:off + KB], pt[:, off:off + KB], mask_t[:]
                    )
                if i + 1 < len(plan):
                    scB = emit_scores_half(*plan[i + 1], 1)
                if len(pending) >= PV_DELAY:
                    emit_pv_and_norm(pending.pop(0))
                pending.append((p, qc, kb, pt, ctx_tiles, pair_tiles[p][2]))
            for rec in pending:
                emit_pv_and_norm(rec)
    nc.compile()
    return nc


def _prep_inputs(query_layer, key_layer, value_layer):
    q = np.asarray(query_layer, dtype=np.float32).reshape(NPAIRS, S, D)
    k = np.asarray(key_layer, dtype=np.float32).reshape(NPAIRS, S, D)
    v = np.asarray(value_layer, dtype=np.float32).reshape(NPAIRS, S, D)

    qt = np.ascontiguousarray(q.transpose(0, 2, 1)).astype(ml_dtypes.bfloat16)
    kt = np.ascontiguousarray(k.transpose(0, 2, 1)).astype(ml_dtypes.bfloat16)
    va = np.ones((NPAIRS, KB, NKT, KB + 1), dtype=ml_dtypes.bfloat16)
    va[:, :, :, :D] = (
        v.reshape(NPAIRS, NKT, KB, D).transpose(0, 2, 1, 3).astype(ml_dtypes.bfloat16)
    )
    in_maps = [
        {
            "qt": np.ascontiguousarray(qt[c * PPC:(c + 1) * PPC]),
            "kt": np.ascontiguousarray(kt[c * PPC:(c + 1) * PPC]),
            "va": np.ascontiguousarray(va[c * PPC:(c + 1) * PPC]),
        }
        for c in range(NCORES)
    ]
    return in_maps


def _run(query_layer, key_layer, value_layer, trace=False):
    in_maps = _prep_inputs(query_layer, key_layer, value_layer)
    nc = _build_nc()
    res = run_bass_kernel_spmd(nc, in_maps, list(range(NCORES)), trace=trace)
    ctx = np.stack([res.results[c]["out"] for c in range(NCORES)])  # [8, PPC, S, D]
    out = ctx.reshape(B, H, S, D).transpose(0, 2, 1, 3).reshape(B, S, H * D)
    return np.ascontiguousarray(out, dtype=np.float32), res


def kernel(query_layer, key_layer, value_layer):
    out, _ = _run(query_layer, key_layer, value_layer, trace=False)
    return out


# revision 41
# speedup vs baseline: 1.0049x; 1.0049x over previous
"""Causal multi-head attention (B=2, H=16, S=2048, D=128, fp32) on 8 NeuronCores.

Sharding: the 32 (batch, head) pairs are split 4-per-core (tensor parallel over
heads, data parallel over batch — both collapse to the fused pair axis).

Per-core kernel, flash-attention style without max-subtraction (scores have
unit variance after the 1/sqrt(D) scale).  All exponentials carry a uniform
shift exp(s - CSHIFT), which softmax normalization cancels.

One flat pipeline over 96 score strips (pair, chunk, k-block).  The exp of
each strip is column-split across BOTH elementwise engines, and each engine
has its OWN score PSUM rotation so neither can stall the other:

  scores_T[k, 0:512]    -> scA (1-bank PSUM, pool A)   exp'd by ScalarE (Exp)
  scores_T[k, 512:1024] -> scB (1-bank PSUM, pool B)   exp'd by DVE via a
      Schraudolph bit-trick: t = rne_int16(s*A + B); bitcast(t) is bf16
      ~ exp(s*SCALE - C) with ~3.3% max rel err (mean-free after softmax)

  pool A's buffer rotation depends only on ScalarE's exp, pool B's only on
  DVE's, and PE's in-order queue is arranged as
      [scores_A(i+1)] [PV batch(i-1)] [scores_B(i+1)]
  so the PV matmuls (which by then have no unresolved deps) stream at full
  rate and a lag in either exp engine delays only its own half.

  causal mask on diagonal 128x128 blocks     (DVE multiply by a const mask)
  ctx[q, 0:128], l[q] = P_T_blk.T @ [V | 1]  (bf16 matmuls, PSUM-accumulated;
                                              the ones column gives the
                                              softmax denominator for free;
                                              emitted PV_DELAY strips late so
                                              they never carry unresolved
                                              deps into PE's in-order queue)
  out[q, :] = ctx[q, :] / l[q]               (DVE batched reciprocal per PSUM
                                              bank + ScalarE Copy-with-scale)

All input DMAs are issued up-front on the (otherwise idle) GPSIMD trigger
queue in consumption order; output DMAs go on the Sync queue.  PSUM layout is
exactly 8 banks: scA x2, scB x2, ctx0, ctx1, ctx2 x2 (double-buffered).
PSUM start=True clears has_written for a whole bank, so of the 8 packed ctx
accumulation groups only the first per bank (s = 0/3/6 at kb==0) uses it.

Q^T / K^T (bf16) and the bf16 [V | 1] augmentation are prepared host-side in
kernel() — host preprocessing is part of the sharding step.
"""

import math

import ml_dtypes
import numpy as np

import concourse.bass as bass
import concourse.mybir as mybir
from concourse import bacc, tile
from concourse.bass_utils import run_bass_kernel_spmd

B, H, S, D = 2, 16, 2048, 128
NCORES = 8
NPAIRS = B * H              # 32 fused (batch, head) pairs
PPC = NPAIRS // NCORES      # 4 pairs per core
KB = 128                    # k block (PE contraction / partition dim)
QC = 1024                   # q chunk (2 half-strips)
HC = 512                    # half-strip columns (one PSUM bank)
NSUB = QC // 128            # sub-q blocks (PV stationary width) per chunk
NKT = S // KB               # 16 k blocks per sequence
SCALE = 1.0 / math.sqrt(D)  # net score scale: /(sqrt(d)*coeff) then *coeff
CSHIFT = 1.25               # uniform exponent shift (cancels in softmax)

# Schraudolph constants: bf16(int16_rne(s_raw*A + B)) ~ exp(s_raw*SCALE - C)
_DELTA = math.log2((1 + (1 / math.log(2) - 1)) / 2 ** (1 / math.log(2) - 1)) / 2
A_SCH = 128 * math.log2(math.e) * SCALE
B_SCH = 128 * 127 - 128 * _DELTA - CSHIFT * 128 * math.log2(math.e)

# ctx bank groups normalized on ScalarE instead of DVE (bank index 0/1/2)
NORM_ACT_BANKS = {0, 1}

F32 = mybir.dt.float32
BF16 = mybir.dt.bfloat16
I16 = mybir.dt.int16


def _build_nc():
    nc = bacc.Bacc("TRN2", target_bir_lowering=False, debug=False)
    qt_d = nc.dram_tensor("qt", [PPC, D, S], BF16, kind="ExternalInput")
    kt_d = nc.dram_tensor("kt", [PPC, D, S], BF16, kind="ExternalInput")
    va_d = nc.dram_tensor("va", [PPC, KB, NKT, KB + 1], BF16, kind="ExternalInput")
    out_d = nc.dram_tensor("out", [PPC, S, D], F32, kind="ExternalOutput")

    # Raw-bass warmup activation before the Tile body: bacc's table-load
    # placement then puts the ~1.3us ACT table load in the preamble, off the
    # first strip's critical path. Persistent scratch; address never reused.
    warm_sb = nc.alloc_sbuf_tensor("warm_sb", [128, 1], F32)
    nc.scalar.activation(
        warm_sb.ap(), warm_sb.ap(), mybir.ActivationFunctionType.Exp, scale=0.0
    )

    # chunk order per pair: last pair does its big chunk first so the kernel
    # tail is the small chunk's short PV backlog
    qcs_of = [[0, 1] if p < PPC - 1 else [1, 0] for p in range(PPC)]

    def nkb_of(qc):
        return (qc * QC + QC) // KB

    with tile.TileContext(nc) as tc:
        with (
            tc.tile_pool(name="cm", bufs=1) as c_pool,
            tc.tile_pool(name="qk", bufs=3) as qk_pool,
            tc.tile_pool(name="vp", bufs=3) as v_pool,
            tc.tile_pool(name="pp", bufs=6) as p_pool,
            tc.tile_pool(name="oo", bufs=8) as o_pool,
            tc.tile_pool(name="rr", bufs=8) as r_pool,
            tc.tile_pool(name="ps_a", bufs=2, space="PSUM") as ps_a,
            tc.tile_pool(name="ps_b", bufs=2, space="PSUM") as ps_b,
            tc.tile_pool(name="ps_c", bufs=1, space="PSUM") as ps_c,
            tc.tile_pool(name="ps_c2", bufs=2, space="PSUM") as ps_c2,
        ):
            # shared causal keep-mask for diagonal blocks: m[i,j]=1 iff j>=i
            mask_t = c_pool.tile([KB, KB], BF16, name="mask_t")
            nc.gpsimd.memset(mask_t[:], 1.0)
            nc.gpsimd.affine_select(
                out=mask_t[:],
                in_=mask_t[:],
                compare_op=mybir.AluOpType.is_ge,
                fill=0.0,
                base=0,
                pattern=[[1, KB]],
                channel_multiplier=-1,
            )
            bias_t = c_pool.tile([KB, 1], F32, name="bias_t")
            nc.gpsimd.memset(bias_t[:], -CSHIFT)

            # all input DMAs up-front on the gpsimd trigger queue, in
            # consumption order (the queue blocks on pool-buffer reuse, which
            # is fine — nothing else runs on gpsimd)
            pair_tiles = []
            for p in range(PPC):
                qt_t = qk_pool.tile([D, S], BF16, tag="qt", name="qt_t")
                kt_t = qk_pool.tile([D, S], BF16, tag="kt", name="kt_t")
                va_t = v_pool.tile([KB, NKT, KB + 1], BF16, tag="va", name="va_t")
                if p == 0:
                    # fine-grained staging so the first scores matmuls (which
                    # need only kt[:, 0:256] / qt[:, 0:1024]) start ASAP; the
                    # first pieces ride the Sync queue, which finishes its
                    # preamble while gpsimd is still building masks
                    qcp = qcs_of[0][0]
                    q0 = qcp * QC
                    nc.sync.dma_start(
                        out=kt_t[:, 0:2 * KB], in_=kt_d[p][:, 0:2 * KB]
                    )
                    nc.sync.dma_start(
                        out=qt_t[:, q0:q0 + QC], in_=qt_d[p][:, q0:q0 + QC]
                    )
                    nc.sync.dma_start(
                        out=kt_t[:, 2 * KB:QC], in_=kt_d[p][:, 2 * KB:QC]
                    )
                    nc.sync.dma_start(
                        out=va_t[:, 0:4], in_=va_d[p][:, 0:4]
                    )
                    if qcp == 0:
                        nc.gpsimd.dma_start(out=qt_t[:, QC:], in_=qt_d[p][:, QC:])
                    nc.gpsimd.dma_start(out=kt_t[:, QC:], in_=kt_d[p][:, QC:])
                    nc.gpsimd.dma_start(out=va_t[:, 4:], in_=va_d[p][:, 4:])
                    pair_tiles.append((qt_t, kt_t, va_t))
                    continue
                for qcp in qcs_of[p]:
                    c0, c1 = qcp * QC, (qcp + 1) * QC
                    nc.gpsimd.dma_start(out=kt_t[:, c0:c1], in_=kt_d[p][:, c0:c1])
                    nc.gpsimd.dma_start(out=qt_t[:, c0:c1], in_=qt_d[p][:, c0:c1])
                if qcs_of[p][0] == 0:
                    kbm = QC // KB
                    nc.gpsimd.dma_start(out=va_t[:, 0:kbm], in_=va_d[p][:, 0:kbm])
                    nc.gpsimd.dma_start(out=va_t[:, kbm:], in_=va_d[p][:, kbm:])
                else:
                    nc.gpsimd.dma_start(out=va_t[:], in_=va_d[p])
                pair_tiles.append((qt_t, kt_t, va_t))

            plan = [
                (p, qc, kb)
                for p in range(PPC)
                for qc in qcs_of[p]
                for kb in range(nkb_of(qc))
            ]

            def lo_of(qc, kb):
                return max(kb * KB - qc * QC, 0)

            def emit_scores_half(p, qc, kb, half):
                """Scores MM for one 512-col half; returns its PSUM tile
                (None if the half is fully causally dead)."""
                lo = lo_of(qc, kb)
                c0, c1 = half * HC, half * HC + HC
                if lo >= c1:
                    return None
                qt_t, kt_t, _ = pair_tiles[p]
                q0 = qc * QC
                k0 = kb * KB
                pool = ps_a if half == 0 else ps_b
                sc = pool.tile([KB, HC], F32, tag="sc", name="sc")
                cc = max(c0, lo)
                nc.tensor.matmul(
                    sc[:, cc - c0:],
                    kt_t[:, k0:k0 + KB],
                    qt_t[:, q0 + cc:q0 + c1],
                    start=True,
                    stop=True,
                )
                return sc

            def emit_pv_and_norm(rec):
                # PV matmuls + normalize for a strip, emitted one strip LATE:
                # by now its exp/mask are long finished, so these matmuls
                # never stall PE's in-order queue ahead of the next scores.
                p, qc, kb, pt, ctx_tiles, va_t = rec
                q0 = qc * QC
                off = kb * KB - q0

                def ctx_ap(s):
                    t, ii = divmod(s, 3)
                    return ctx_tiles[t][:, ii, :]

                s_order = [s for s in range(NSUB) if off <= s * 128]
                if off >= 0 and kb > 0 and s_order[0] * 128 == off:
                    s_order = s_order[1:] + s_order[:1]
                for s in s_order:
                    qs0 = s * 128
                    nc.tensor.matmul(
                        ctx_ap(s),
                        pt[:, qs0:qs0 + 128],
                        va_t[:, kb, :],
                        start=(kb == 0 and s % 3 == 0),
                        stop=(kb == q0 // KB + s),
                        skip_group_check=True,
                    )
                for bank, s_hi in ((0, 2), (1, 5), (2, 7)):
                    if kb != q0 // KB + s_hi:
                        continue
                    s_lo = 3 * bank
                    nsb = s_hi - s_lo + 1
                    ob = o_pool.tile([128, 3, D], F32, tag="ob", name="ob")
                    rec_t = r_pool.tile([128, 3], F32, tag="rec", name="rec_t")
                    nc.vector.reciprocal(
                        rec_t[:, 0:nsb], ctx_tiles[bank][:, 0:nsb, D]
                    )
                    if bank in NORM_ACT_BANKS:
                        for s in range(s_lo, s_hi + 1):
                            j = s - s_lo
                            nc.scalar.activation(
                                ob[:, j, :],
                                ctx_ap(s)[:, 0:D],
                                mybir.ActivationFunctionType.Copy,
                                scale=rec_t[:, j:j + 1],
                            )
                    else:
                        nc.vector.scalar_tensor_tensor(
                            out=ob[:, 0:nsb, :],
                            in0=ctx_tiles[bank][:, 0:nsb, 0:D],
                            scalar=1.0,
                            in1=rec_t[:, 0:nsb].to_broadcast((128, nsb, D)),
                            op0=mybir.AluOpType.mult,
                            op1=mybir.AluOpType.mult,
                        )
                    nc.sync.dma_start(
                        out=out_d[
                            p, q0 + s_lo * 128:q0 + (s_hi + 1) * 128, :
                        ].rearrange("(s q) d -> q s d", s=nsb),
                        in_=ob[:, 0:nsb, :],
                    )

            scA = emit_scores_half(*plan[0], 0)
            scB = emit_scores_half(*plan[0], 1)
            ctx_tiles = None
            pending = []  # PV batches run PV_DELAY strips late
            PV_DELAY = 5
            small_flip = 0
            for i, (p, qc, kb) in enumerate(plan):
                q0 = qc * QC
                off = kb * KB - q0  # >= 0 on diagonal strips
                lo = max(off, 0)
                if kb == 0:
                    # 8 ctx accumulators [128q, 129] for this chunk, packed
                    # 3/3/2 into PSUM banks
                    ctx_tiles = [
                        ps_c.tile([128, 3, KB + 1], F32, tag="ctx0", name="ctx0"),
                        ps_c.tile([128, 3, KB + 1], F32, tag="ctx1", name="ctx1"),
                        ps_c2.tile([128, 2, KB + 1], F32, tag="ctx2", name="ctx2"),
                    ]

                pt = p_pool.tile([KB, QC], BF16, tag="pt", bufs=10, name="pt")

                def exp_act(sc, g0, g1, pt=pt):
                    # ScalarE exp of global cols [g0:g1) from half tile sc
                    h0 = (g0 // HC) * HC
                    nc.scalar.activation(
                        pt[:, g0:g1],
                        sc[:, g0 - h0:g1 - h0],
                        mybir.ActivationFunctionType.Exp,
                        scale=SCALE,
                        bias=bias_t[:],
                    )

                def exp_dve(sc, g0, g1, pt=pt):
                    h0 = (g0 // HC) * HC
                    nc.vector.tensor_scalar(
                        out=pt[:, g0:g1].bitcast(I16),
                        in0=sc[:, g0 - h0:g1 - h0],
                        scalar1=A_SCH,
                        scalar2=B_SCH,
                        op0=mybir.AluOpType.mult,
                        op1=mybir.AluOpType.add,
                    )

                if lo < HC:
                    exp_act(scA, lo, HC)
                    exp_dve(scB, HC, QC)
                elif small_flip == 0:
                    exp_act(scB, lo, QC)
                    small_flip = 1
                else:
                    exp_dve(scB, lo, QC)
                    small_flip = 0

                # next strip's half-A scores feed ScalarE as soon as its pool
                # slot frees; half-B goes after the PV batch so a DVE lag
                # never blocks the PV stream in PE's in-order queue
                if i + 1 < len(plan):
                    scA = emit_scores_half(*plan[i + 1], 0)

                if off >= 0:
                    # diagonal 128x128 block: keep j >= i, zero rest
                    nc.vector.tensor_mul(
                        pt[:, off:off + KB], pt[:, off:off + KB], mask_t[:]
                    )
                if i + 1 < len(plan):
                    scB = emit_scores_half(*plan[i + 1], 1)
                if len(pending) >= PV_DELAY:
                    emit_pv_and_norm(pending.pop(0))
                pending.append((p, qc, kb, pt, ctx_tiles, pair_tiles[p][2]))
            for rec in pending:
                emit_pv_and_norm(rec)
    nc.compile()
    return nc


def _prep_inputs(query_layer, key_layer, value_layer):
    q = np.asarray(query_layer, dtype=np.float32).reshape(NPAIRS, S, D)
    k = np.asarray(key_layer, dtype=np.float32).reshape(NPAIRS, S, D)
    v = np.asarray(value_layer, dtype=np.float32).reshape(NPAIRS, S, D)

    qt = np.ascontiguousarray(q.transpose(0, 2, 1)).astype(ml_dtypes.bfloat16)
    kt = np.ascontiguousarray(k.transpose(0, 2, 1)).astype(ml_dtypes.bfloat16)
    va = np.ones((NPAIRS, KB, NKT, KB + 1), dtype=ml_dtypes.bfloat16)
    va[:, :, :, :D] = (
        v.reshape(NPAIRS, NKT, KB, D).transpose(0, 2, 1, 3).astype(ml_dtypes.bfloat16)
    )
    in_maps = [
        {
            "qt": np.ascontiguousarray(qt[c * PPC:(c + 1) * PPC]),
            "kt": np.ascontiguousarray(kt[c * PPC:(c + 1) * PPC]),
            "va": np.ascontiguousarray(va[c * PPC:(c + 1) * PPC]),
        }
        for c in range(NCORES)
    ]
    return in_maps


def _run(query_layer, key_layer, value_layer, trace=False):
    in_maps = _prep_inputs(query_layer, key_layer, value_layer)
    nc = _build_nc()
    res = run_bass_kernel_spmd(nc, in_maps, list(range(NCORES)), trace=trace)
    ctx = np.stack([res.results[c]["out"] for c in range(NCORES)])  # [8, PPC, S, D]
    out = ctx.reshape(B, H, S, D).transpose(0, 2, 1, 3).reshape(B, S, H * D)
    return np.ascontiguousarray(out, dtype=np.float32), res


def kernel(query_layer, key_layer, value_layer):
    out, _ = _run(query_layer, key_layer, value_layer, trace=False)
    return out
